# revision 1
# baseline (speedup 1.0000x reference)
"""Adaptive bilateral filter (nn_AdaptiveFilter) on 8 TRN2 NeuronCores.

Math: out_c(p) = sum_k x_c(p+d_k) * wt_k(p) / sum_k wt_k(p)
with wt_k = softmax_k(w)(p) * exp(-50 * (sum_c |g_c(p+d_k) - g_c(p)|)^2).
Softmax normalization (and its max-subtraction) cancels in num/den, so
wt_k = exp(w0[src(k)]) * exp(-50*s^2) with src = reflect map (7,7)->(4,4).

Sharding: 8 cores = 2 batches x 4 row-bands of 128 rows. Host reflect-pads
to (518,518) and ships each core a (3,134,518) band of x and guidance (halo
included), plus its (128, 512*16) slice of w0. No collectives.

Engine split (per tap-row i, j-packed over the 7 column taps):
  DVE:    3 bf16 subtracts (sliding-window in0 vs broadcast center in1),
          3 bitwise-AND abs (uint16 bitcast, 4x mode), wt = col*E
          (j-packed via +-512-stride views into the E bank), 3 x*wt products
  ACT:    f32->bf16 input conversions, 16 exp(w0) at setup, and per tap ONE
          Derivative_Erf(sqrt(50)*s) = 2/sqrt(pi)*exp(-50 s^2) straight from
          PSUM (the 2/sqrt(pi) cancels between num and den)
  PE:     channel-sum of |d| into PSUM (identity matmuls), den/num
          accumulation over the 49 taps into PSUM banks
  GPSIMD: nothing (measured far slower than nominal for wide tensor ops)
"""
import sys
sys.path.insert(0, "/opt/trn_rl_repo")
import math
import numpy as np

import concourse.bacc as bacc
import concourse.mybir as mybir
import concourse.tile as tile
from concourse.ap import AP
from concourse.bass_utils import run_bass_kernel_spmd

F32 = mybir.dt.float32
BF16 = mybir.dt.bfloat16
U16 = mybir.dt.uint16
AF = mybir.ActivationFunctionType
OP = mybir.AluOpType

KH = KW = 7
H_BAND = 128
W = 512
WP = 518
WJ = KW * W  # 3584
SCALE = math.sqrt(50.0)  # Square(sqrt(50)*s) = 50*s^2

_CACHE = {}


def _win(ap_obj, nwin, wsize):
    """[128, C] SBUF AP -> [128, nwin, wsize] overlapping windows (step 1)."""
    base = ap_obj.ap
    assert base[-1][0] == 1
    return AP(tensor=ap_obj.tensor, offset=ap_obj.offset,
              ap=[list(base[0]), [1, nwin], [1, wsize]])


def _emit(nc, tc, constp, gxp, workp, finp, psump, g_d, x_d, w_d, id_d, out_d):
    ident = constp.tile([128, 128], BF16, tag="ident", name="ident")
    nc.sync.dma_start(ident[:], id_d.ap()[:, :])

    # E[:, t*512:(t+1)*512] = exp(w0 source tap t), t = ti*4+tj  (bf16)
    E = constp.tile([H_BAND, 16 * W], BF16, tag="E", name="E")
    with tc.tile_pool(name="wpool", bufs=1) as wpool:
        HW2 = W // 2
        for h in range(2):
            wraw = wpool.tile([H_BAND, HW2 * 16], F32, tag="wraw", name="wraw")
            nc.sync.dma_start(wraw[:], w_d.ap()[:, h * HW2 * 16:(h + 1) * HW2 * 16])
            wv = wraw[:].rearrange("p (w s) -> p s w", s=16)
            for t in range(16):
                nc.scalar.activation(
                    E[:, t * W + h * HW2:t * W + (h + 1) * HW2], wv[:, t, :], AF.Exp)

    # center guidance (shift i=3, cols 3..514), bf16
    gcb = []
    for ch in range(3):
        tf = constp.tile([H_BAND, W], F32, tag=f"gcf{ch}", name=f"gcf{ch}")
        nc.sync.dma_start(tf[:], g_d.ap()[ch, 3:3 + H_BAND, 3:3 + W])
        tb = constp.tile([H_BAND, W], BF16, tag=f"gc{ch}", name=f"gc{ch}")
        nc.scalar.copy(tb[:], tf[:])
        gcb.append(tb)

    den_ps = psump.tile([H_BAND, W], F32, tag="dps", name="dps", bufs=1)
    num_wide = psump.tile([H_BAND, 3 * W], F32, tag="npsw", name="npsw", bufs=1)
    num_ps = [num_wide[:, c * W:(c + 1) * W] for c in range(3)]

    for i in range(KH):
        gib, xib = [], []
        for ch in range(3):
            tf = gxp.tile([H_BAND, WP], F32, tag=f"gf{ch}", name=f"gf{ch}", bufs=2)
            nc.sync.dma_start(tf[:], g_d.ap()[ch, i:i + H_BAND, :])
            tb = gxp.tile([H_BAND, WP], BF16, tag=f"gb{ch}", name=f"gb{ch}", bufs=3)
            nc.scalar.copy(tb[:], tf[:])
            gib.append(tb)
        for ch in range(3):
            tf = gxp.tile([H_BAND, WP], F32, tag=f"xf{ch}", name=f"xf{ch}", bufs=2)
            nc.sync.dma_start(tf[:], x_d.ap()[ch, i:i + H_BAND, :])
            tb = gxp.tile([H_BAND, WP], BF16, tag=f"xb{ch}", name=f"xb{ch}", bufs=3)
            nc.scalar.copy(tb[:], tf[:])
            xib.append(tb)

        ri = min(i, 6 - i)
        first_i, last_i = (i == 0), (i == 6)

        # u_c = g window - center (bf16, j-packed), then |u_c| via sign-bit AND
        ab = []
        for ch in range(3):
            u = workp.tile([H_BAND, WJ], BF16, tag=f"u{ch}", name=f"u{ch}", bufs=3)
            uv = u[:].rearrange("p (n w) -> p n w", n=KW)
            nc.vector.tensor_tensor(
                uv, _win(gib[ch][:, :], KW, W),
                gcb[ch][:, :].unsqueeze(1).broadcast_to([H_BAND, KW, W]),
                OP.subtract)
            nc.vector.tensor_scalar(u[:].bitcast(U16), u[:].bitcast(U16),
                                    0x7FFF, None, OP.bitwise_and)
            ab.append(u)

        wt_wide = workp.tile([H_BAND, WJ], BF16, tag="wt", name="wt", bufs=2)
        col_wide = workp.tile([H_BAND, WJ], BF16, tag="colw", name="colw", bufs=2)
        for j in range(KW):
            # s = sum_c |u_c| via PE accumulation (PSUM f32)
            s_ps = psump.tile([H_BAND, W], F32, tag="sps", name="sps", bufs=4)
            for ch in range(3):
                nc.tensor.matmul(s_ps[:], ident[:], ab[ch][:, j * W:(j + 1) * W],
                                 start=(ch == 0), stop=(ch == 2))
            # Derivative_Erf(sqrt(50)*s) = 2/sqrt(pi) * exp(-50*s^2); the
            # 2/sqrt(pi) factor cancels between num and den.
            nc.scalar.activation(col_wide[:, j * W:(j + 1) * W], s_ps[:],
                                 AF.Derivative_Erf, scale=SCALE)

        # wt = col * E(src tap): j in 0..3 reads E slots 4ri..4ri+3 (step +W),
        # j in 4..6 reads slots 4ri+2..4ri (step -W)
        ebase = E[:].offset
        up = AP(tensor=E[:].tensor, offset=ebase + (4 * ri) * W,
                ap=[[16 * W, H_BAND], [W, 4], [1, W]])
        dn = AP(tensor=E[:].tensor, offset=ebase + (4 * ri + 2) * W,
                ap=[[16 * W, H_BAND], [-W, 3], [1, W]])
        nc.vector.tensor_tensor(
            wt_wide[:, 0:4 * W].rearrange("p (n w) -> p n w", n=4),
            col_wide[:, 0:4 * W].rearrange("p (n w) -> p n w", n=4), up, OP.mult)
        nc.vector.tensor_tensor(
            wt_wide[:, 4 * W:].rearrange("p (n w) -> p n w", n=3),
            col_wide[:, 4 * W:].rearrange("p (n w) -> p n w", n=3), dn, OP.mult)
        for j in range(KW):
            nc.tensor.matmul(den_ps[:], ident[:], wt_wide[:, j * W:(j + 1) * W],
                             start=(first_i and j == 0), stop=(last_i and j == 6))

        # products and num accumulation (j-packed); ch2 on GPSIMD
        for ch in range(3):
            prod = workp.tile([H_BAND, WJ], BF16, tag=f"pr{ch}", name=f"pr{ch}",
                              bufs=1)
            pv = prod[:].rearrange("p (n w) -> p n w", n=KW)
            eng = nc.vector
            eng.tensor_tensor(
                pv, _win(xib[ch][:, :], KW, W),
                wt_wide[:].rearrange("p (n w) -> p n w", n=KW), OP.mult)
            for j in range(KW):
                nc.tensor.matmul(num_ps[ch], ident[:],
                                 prod[:, j * W:(j + 1) * W],
                                 start=(first_i and j == 0),
                                 stop=(last_i and j == 6))

    rec = finp.tile([H_BAND, W], F32, tag="rec", name="rec")
    # den in [~4e-3, ~60]: approx_fast's ~51 ULP is negligible vs bf16 noise
    nc.vector.reciprocal_approx_fast(rec[:], den_ps[:])
    o = finp.tile([H_BAND, 3 * W], F32, tag="ow", name="ow")
    nc.vector.tensor_tensor(
        o[:].rearrange("p (c w) -> p c w", c=3),
        num_wide[:].rearrange("p (c w) -> p c w", c=3),
        rec[:].unsqueeze(1).broadcast_to([H_BAND, 3, W]), OP.mult)
    od = out_d.ap()
    dst = AP(tensor=od.tensor, offset=od.offset,
             ap=[[W, H_BAND], [H_BAND * W, 3], [1, W]])
    nc.sync.dma_start(dst, o[:].rearrange("p (c w) -> p c w", c=3))


def _build(reps=1, loop_n=None):
    nc = bacc.Bacc("TRN2", target_bir_lowering=False, debug=False)
    g_d = nc.dram_tensor("g", [3, 134, WP], F32, kind="ExternalInput")
    x_d = nc.dram_tensor("x", [3, 134, WP], F32, kind="ExternalInput")
    w_d = nc.dram_tensor("w", [H_BAND, W * 16], F32, kind="ExternalInput")
    id_d = nc.dram_tensor("ident", [128, 128], BF16, kind="ExternalInput")
    out_d = nc.dram_tensor("out", [3, H_BAND, W], F32, kind="ExternalOutput")

    with tile.TileContext(nc) as tc:
        with (
            tc.tile_pool(name="const", bufs=1) as constp,
            tc.tile_pool(name="gx", bufs=2) as gxp,
            tc.tile_pool(name="work", bufs=2) as workp,
            tc.tile_pool(name="fin", bufs=1) as finp,
            tc.tile_pool(name="psum", bufs=1, space="PSUM") as psump,
        ):
            if loop_n is not None:
                with tc.For_i(0, loop_n, 1):
                    _emit(nc, tc, constp, gxp, workp, finp, psump,
                          g_d, x_d, w_d, id_d, out_d)
            else:
                for _rep in range(reps):
                    _emit(nc, tc, constp, gxp, workp, finp, psump,
                          g_d, x_d, w_d, id_d, out_d)

    nc.compile()
    return nc


def _shard_inputs(x, guidance, w0):
    import ml_dtypes
    pad = ((0, 0), (0, 0), (3, 3), (3, 3))
    xp = np.pad(x, pad, mode="reflect")
    gp = np.pad(guidance, pad, mode="reflect")
    ident = np.eye(128, dtype=ml_dtypes.bfloat16)

    in_maps = []
    for c in range(8):
        b, band = divmod(c, 4)
        r0 = band * H_BAND
        in_maps.append({
            "g": np.ascontiguousarray(gp[b, :, r0:r0 + H_BAND + 6, :]),
            "x": np.ascontiguousarray(xp[b, :, r0:r0 + H_BAND + 6, :]),
            "w": np.ascontiguousarray(
                w0[b, r0 * W:(r0 + H_BAND) * W].reshape(H_BAND, W * 16)),
            "ident": ident,
        })
    return in_maps


def kernel(x, guidance, w0):
    x = np.asarray(x, dtype=np.float32)
    guidance = np.asarray(guidance, dtype=np.float32)
    w0 = np.asarray(w0, dtype=np.float32)
    B, C, H, Wf = x.shape

    if "nc" not in _CACHE:
        _CACHE["nc"] = _build()
    nc = _CACHE["nc"]

    in_maps = _shard_inputs(x, guidance, w0)
    res = run_bass_kernel_spmd(nc, in_maps, core_ids=list(range(8)))

    out = np.empty((B, C, H, Wf), dtype=np.float32)
    for c in range(8):
        b, band = divmod(c, 4)
        r0 = band * H_BAND
        out[b, :, r0:r0 + H_BAND, :] = res.results[c]["out"]
    return out



# revision 7
# speedup vs baseline: 1.3226x; 1.3226x over previous
"""Adaptive bilateral filter (nn_AdaptiveFilter) on 8 TRN2 NeuronCores.

Math: out_c(p) = sum_k x_c(p+d_k) * wt_k(p) / sum_k wt_k(p)
with wt_k = softmax_k(w)(p) * exp(-50 * (sum_c |g_c(p+d_k) - g_c(p)|)^2).
Softmax normalization cancels in num/den, so wt_k = E[src(k)] * exp(-50*s^2)
with E = exp(w0) precomputed on HOST (slot-major bf16) and src = reflect
map (7,7)->(4,4).

Sharding: 8 cores = 2 batches x 4 row-bands of 128 rows. Host reflect-pads
to (518,518), converts g/x to bf16, and ships each core a (3,134,518) bf16
band (halo included), E [128, 16*512] bf16, and the center-guidance tile
gc [128, 3*512] bf16. No collectives.

Engine split per tap-row i (j-packed over 7 column taps, c-packed over 3
channels):
  DVE:    ONE bf16 subtract [128, 3*7*512] (sliding-window in0 vs broadcast
          center in1), wt = col*E (two ops, +-512-stride E views), ONE
          product x*wt [128, 3*7*512]
  ACT:    in-place Abs on the subtract output, per tap ONE
          Derivative_Erf(sqrt(50)*s) = 2/sqrt(pi)*exp(-50 s^2) from PSUM
          (the 2/sqrt(pi) cancels between num and den)
  PE:     channel-sum of |d| into PSUM (identity matmuls), den/num
          accumulation over the 49 taps; j-groups folded into single
          matmuls via stride-0 output APs (PSUM accumulates per write)
Emission is software-pipelined (stage A: dma+sub+abs at i, stage B:
s-matmuls+DErf at i-1, stage C: wt+den+prod+num at i-2) so no engine
queue blocks on a cross-engine dependency.
"""
import sys
sys.path.insert(0, "/opt/trn_rl_repo")
import math
import numpy as np

import concourse.bacc as bacc
import concourse.mybir as mybir
import concourse.tile as tile
from concourse.ap import AP
from concourse.bass_utils import run_bass_kernel_spmd

F32 = mybir.dt.float32
BF16 = mybir.dt.bfloat16
AF = mybir.ActivationFunctionType
OP = mybir.AluOpType

KH = KW = 7
H_BAND = 128
W = 512
WP = 518
WJ = KW * W        # 3584
CJ = 3 * WJ        # 10752
SCALE = math.sqrt(50.0)  # Square(sqrt(50)*s) = 50*s^2

_CACHE = {}


def _view(ap_obj, dims):
    """AP with the tile's partition dim plus the given free [stride, size]."""
    base = ap_obj.ap
    return AP(tensor=ap_obj.tensor, offset=ap_obj.offset,
              ap=[list(base[0])] + [list(d) for d in dims])


def _emit(nc, tc, constp, gxp, workp, finp, psump, g_d, x_d, e_d, gc_d,
          id_d, out_d):
    ident = constp.tile([128, 128], BF16, tag="ident", name="ident")
    nc.sync.dma_start(ident[:], id_d.ap()[:, :])

    gc = constp.tile([H_BAND, 3 * W], BF16, tag="gc", name="gc")
    nc.sync.dma_start(gc[:], gc_d.ap()[:, :])

    # E chunk t holds source-row ti=t taps (slots 4t..4t+3); DMA'd inside
    # loop iter t so g0/x0 aren't stuck behind 2 MB of E in the queue.
    E = [constp.tile([H_BAND, 4 * W], BF16, tag=f"E{t}", name=f"E{t}")
         for t in range(4)]

    den_ps = psump.tile([H_BAND, W], F32, tag="dps", name="dps", bufs=1)
    num_wide = psump.tile([H_BAND, 3 * W], F32, tag="npsw", name="npsw",
                          bufs=1)

    stageA = {}
    stageB = {}

    def emit_A(i):
        gt = gxp.tile([H_BAND, 3 * WP], BF16, tag="gt", name="gt", bufs=2)
        nc.sync.dma_start(
            gt[:].rearrange("p (c w) -> p c w", c=3),
            g_d.ap()[:, i:i + H_BAND, :].rearrange("c h w -> h c w"))
        xt = gxp.tile([H_BAND, 3 * WP], BF16, tag="xt", name="xt", bufs=3)
        nc.sync.dma_start(
            xt[:].rearrange("p (c w) -> p c w", c=3),
            x_d.ap()[:, i:i + H_BAND, :].rearrange("c h w -> h c w"))
        if i < 4:
            nc.sync.dma_start(E[i][:], e_d.ap()[:, i * 4 * W:(i + 1) * 4 * W])
        # u[p, c, j, w] = gt[p, c*518 + j + w] - gc[p, c*512 + w]
        u = workp.tile([H_BAND, CJ], BF16, tag="u", name="u", bufs=2)
        nc.vector.tensor_tensor(
            u[:].rearrange("p (c n w) -> p c n w", c=3, n=KW),
            _view(gt[:], [[WP, 3], [1, KW], [1, W]]),
            _view(gc[:], [[W, 3], [0, KW], [1, W]]),
            OP.subtract)
        nc.scalar.activation(u[:], u[:], AF.Abs)
        stageA[i] = (u, xt)

    def emit_B(i):
        u, xt = stageA.pop(i)
        col = workp.tile([H_BAND, WJ], BF16, tag="col", name="col", bufs=2)
        for j in range(KW):
            # s = sum_c |u_c| via PE accumulation (PSUM f32)
            s_ps = psump.tile([H_BAND, W], F32, tag="sps", name="sps", bufs=4)
            for c in range(3):
                nc.tensor.matmul(s_ps[:], ident[:],
                                 u[:, c * WJ + j * W:c * WJ + (j + 1) * W],
                                 start=(c == 0), stop=(c == 2))
            nc.scalar.activation(col[:, j * W:(j + 1) * W], s_ps[:],
                                 AF.Derivative_Erf, scale=SCALE)
        stageB[i] = (col, xt)

    def emit_C(i):
        col, xt = stageB.pop(i)
        ri = min(i, 6 - i)
        first_i, last_i = (i == 0), (i == 6)
        # wt = col * E(src tap): j in 0..3 reads E[ri] slots 0..3 (+W step),
        # j in 4..6 reads slots 2..0 (-W step)
        wt = workp.tile([H_BAND, WJ], BF16, tag="wt", name="wt", bufs=2)
        eb = E[ri][:]
        nc.vector.tensor_tensor(
            wt[:, 0:4 * W].rearrange("p (n w) -> p n w", n=4),
            col[:, 0:4 * W].rearrange("p (n w) -> p n w", n=4),
            _view(eb, [[W, 4], [1, W]]), OP.mult)
        nc.vector.tensor_tensor(
            wt[:, 4 * W:].rearrange("p (n w) -> p n w", n=3),
            col[:, 4 * W:].rearrange("p (n w) -> p n w", n=3),
            AP(tensor=eb.tensor, offset=eb.offset + 2 * W,
               ap=[list(eb.ap[0]), [-W, 3], [1, W]]), OP.mult)
        for j in range(KW):
            nc.tensor.matmul(den_ps[:], ident[:], wt[:, j * W:(j + 1) * W],
                             start=(first_i and j == 0),
                             stop=(last_i and j == 6))
        # prod[p, c, j, w] = xt[p, c*518 + j + w] * wt[p, j*512 + w]
        prod = workp.tile([H_BAND, CJ], BF16, tag="pr", name="pr", bufs=2)
        nc.vector.tensor_tensor(
            prod[:].rearrange("p (c n w) -> p c n w", c=3, n=KW),
            _view(xt[:], [[WP, 3], [1, KW], [1, W]]),
            _view(wt[:], [[0, 3], [W, KW], [1, W]]),
            OP.mult)
        for c in range(3):
            for j in range(KW):
                nc.tensor.matmul(
                    num_wide[:, c * W:(c + 1) * W], ident[:],
                    prod[:, c * WJ + j * W:c * WJ + (j + 1) * W],
                    start=(first_i and j == 0),
                    stop=(last_i and j == 6))

    for i in range(KH):
        emit_A(i)
        if i >= 1:
            emit_B(i - 1)
        if i >= 2:
            emit_C(i - 2)
    emit_B(6)
    emit_C(5)
    emit_C(6)

    rec = finp.tile([H_BAND, W], F32, tag="rec", name="rec")
    # den in [~4e-3, ~60]: approx_fast's ~51 ULP is negligible vs bf16 noise
    nc.vector.reciprocal_approx_fast(rec[:], den_ps[:])
    od = out_d.ap()
    for c in range(3):
        o = finp.tile([H_BAND, W], F32, tag=f"o{c}", name=f"o{c}")
        nc.vector.tensor_tensor(o[:], num_wide[:, c * W:(c + 1) * W],
                                rec[:], OP.mult)
        dst = AP(tensor=od.tensor, offset=od.offset + c * H_BAND * W,
                 ap=[[W, H_BAND], [1, W]])
        nc.sync.dma_start(dst, o[:])


def _build():
    nc = bacc.Bacc("TRN2", target_bir_lowering=False, debug=False)
    g_d = nc.dram_tensor("g", [3, 134, WP], BF16, kind="ExternalInput")
    x_d = nc.dram_tensor("x", [3, 134, WP], BF16, kind="ExternalInput")
    e_d = nc.dram_tensor("e", [H_BAND, 16 * W], BF16, kind="ExternalInput")
    gc_d = nc.dram_tensor("gc", [H_BAND, 3 * W], BF16, kind="ExternalInput")
    id_d = nc.dram_tensor("ident", [128, 128], BF16, kind="ExternalInput")
    out_d = nc.dram_tensor("out", [3, H_BAND, W], F32, kind="ExternalOutput")

    with tile.TileContext(nc) as tc:
        with (
            tc.tile_pool(name="const", bufs=1) as constp,
            tc.tile_pool(name="gx", bufs=2) as gxp,
            tc.tile_pool(name="work", bufs=2) as workp,
            tc.tile_pool(name="fin", bufs=1) as finp,
            tc.tile_pool(name="psum", bufs=1, space="PSUM") as psump,
        ):
            _emit(nc, tc, constp, gxp, workp, finp, psump,
                  g_d, x_d, e_d, gc_d, id_d, out_d)

    nc.compile()
    return nc


def _shard_inputs(x, guidance, w0):
    import ml_dtypes
    BF = ml_dtypes.bfloat16
    pad = ((0, 0), (0, 0), (3, 3), (3, 3))
    xp = np.pad(x, pad, mode="reflect").astype(BF)
    gp = np.pad(guidance, pad, mode="reflect").astype(BF)
    ident = np.eye(128, dtype=BF)

    in_maps = []
    for c in range(8):
        b, band = divmod(c, 4)
        r0 = band * H_BAND
        wslice = w0[b, r0 * W:(r0 + H_BAND) * W]          # (65536, 4, 4)
        e = np.exp(wslice.reshape(H_BAND, W, 4, 4).transpose(0, 2, 3, 1))
        gcore = gp[b, :, 3 + r0:3 + r0 + H_BAND, 3:3 + W]  # (3, 128, 512)
        in_maps.append({
            "g": np.ascontiguousarray(gp[b, :, r0:r0 + H_BAND + 6, :]),
            "x": np.ascontiguousarray(xp[b, :, r0:r0 + H_BAND + 6, :]),
            "e": np.ascontiguousarray(e.reshape(H_BAND, 16 * W)).astype(BF),
            "gc": np.ascontiguousarray(
                gcore.transpose(1, 0, 2).reshape(H_BAND, 3 * W)),
            "ident": ident,
        })
    return in_maps


def kernel(x, guidance, w0):
    x = np.asarray(x, dtype=np.float32)
    guidance = np.asarray(guidance, dtype=np.float32)
    w0 = np.asarray(w0, dtype=np.float32)
    B, C, H, Wf = x.shape

    if "nc" not in _CACHE:
        _CACHE["nc"] = _build()
    nc = _CACHE["nc"]

    in_maps = _shard_inputs(x, guidance, w0)
    res = run_bass_kernel_spmd(nc, in_maps, core_ids=list(range(8)))

    out = np.empty((B, C, H, Wf), dtype=np.float32)
    for c in range(8):
        b, band = divmod(c, 4)
        r0 = band * H_BAND
        out[b, :, r0:r0 + H_BAND, :] = res.results[c]["out"]
    return out
